# revision 64
# baseline (speedup 1.0000x reference)
"""Bahdanau (additive) attention TRN2 Bass kernel — v5, S-on-partitions.

reference:
    proj_in = einsum("bse,ea->bsa", inputs, W_in)      # [B,S,A]
    proj_q  = (query @ W_q)[:, None, :]                # [B,1,A]
    scores  = einsum("bsa,a->bs", tanh(proj_in+proj_q), w_att)
    weights = softmax(scores, axis=1)
    context = einsum("bs,bsa->ba", weights, proj_in)   # [B,A]

B,S,E,Q,A = 32,2048,1024,1024,512.

Sharding: data-parallel over batch. 8 cores x 4 batches; weights
replicated. proj_q is precomputed on the host (0.05% of FLOPs) and
shipped pre-broadcast across partitions, as is w_att. X and W_in are
shipped in device-native tile layouts so every DMA is 128 descriptors
of multi-KB contiguous runs (HWDGE dispatch ~5ns/descriptor, and the
"small descriptors are slow" HBM penalty never hits).

Device algorithm (per batch; proj tiles come out [s_tile=128, a=512]
with the SEQUENCE dim on partitions):
  - per s_tile (16): 8 e-chunk matmuls accumulate psum[s,a]; one DVE
    tensor_tensor drains psum, fusing the +proj_q bias (partition-
    broadcast tile) and the bf16 cast -> pb.
  - ACT tanh(pb) -> t; one DVE scalar_tensor_tensor computes
    (t * w_att_bcast) with accum_out = scores[:,st] (fused
    multiply+free-dim-reduce: softmax scores land ON partitions).
  - ACT exp -> expbf[:,st] bf16 (no max subtraction: |scores|<=~3).
    No cross-partition broadcast anywhere.
  - context: PE matmuls ctx[1,a] += expbf[:,st].T @ pb(st), emitted in
    RUNS OF EIGHT trailing the main stream (each main<->ctx stationary
    switch costs ~190ns on the PE, so grouping beats interleaving
    singly; the lag keeps the PE from head-blocking on the DVE/ACT
    chain). Denominator = ones.T @ expbf + reduce + reciprocal,
    emitted before the ctx run so the DVE chain hides under it.
  - 16 dummy matmuls (realistic 128x128x512 shape; M=1 dummies
    measurably slow all later matmuls) warm the PE HAM clock gate
    1.2->2.4GHz during the initial DMA wait.
  - out_row = (ctx * 1/total) - bf16(projq) in one fused DVE op; since
    pb = proj_in + bf16(projq), subtracting the same bf16 projq
    cancels the bias exactly. Contiguous 2KB output DMA per batch.
  - batch b's last ctx run + finalize interleave into batch b+1's
    stream; only batch 3's ~3us chain is a tail.
  - Input DMAs are explicitly CHAINED (add_dep_helper): concurrent
    DMAs round-robin per-packet across the 16 SDMA engines and all
    finish together at the BW limit, so an unordered head load gates
    the first matmul on the LAST byte of 5MB. The chain staggers
    arrivals: W, then batch-0 X in eighths, then batches 1-3 whole.
"""

import sys

sys.path.insert(0, "/opt/trn_rl_repo")

import ml_dtypes
import numpy as np

import concourse.bass as bass
import concourse.tile as tile
from concourse import bacc, bass_utils, mybir

B, S, E, Q, A = 32, 2048, 1024, 1024, 512
NCORES = 8
BPC = B // NCORES  # batches per core
P = 128
EC = E // P  # 8 e-chunks (contraction)
ST = S // P  # 16 s-tiles per batch (output partition tiles)
NQ = 8  # batch-0 load slices (s-eighths)
QW = S // NQ  # 256
TPQ = ST // NQ  # s_tiles per load slice = 2
CTX_RUN = 8  # ctx matmuls emitted in runs of this many
N_WARMUP = 14  # dummy PE-warmup matmuls before the real stream

BF = mybir.dt.bfloat16
F32 = mybir.dt.float32
TANH = mybir.ActivationFunctionType.Tanh
EXP = mybir.ActivationFunctionType.Exp


def build():
    nc = bacc.Bacc("TRN2", target_bir_lowering=False, debug=False)

    # host-tiled layouts: xh[b, p, q, ec, qw], wh[p, ec, a]
    xh = nc.dram_tensor("xh", [BPC, P, NQ, EC, QW], BF, kind="ExternalInput")
    wh = nc.dram_tensor("wh", [P, EC, A], BF, kind="ExternalInput")
    pq = nc.dram_tensor("pq", [1, BPC * A], BF, kind="ExternalInput")
    wa = nc.dram_tensor("wa", [1, A], BF, kind="ExternalInput")
    out = nc.dram_tensor("out", [BPC, A], F32, kind="ExternalOutput")
    warmout = nc.dram_tensor("warmout", [1, 1], F32, kind="ExternalOutput")

    with tile.TileContext(nc) as tc:
        with (
            tc.tile_pool(name="const", bufs=1) as const,
            tc.tile_pool(name="xtp", bufs=2) as xtp,
            tc.tile_pool(name="pbp", bufs=13) as pbp,
            tc.tile_pool(name="tp", bufs=3) as tp,
            tc.tile_pool(name="small", bufs=2) as small,
            tc.tile_pool(name="mm_ps", bufs=6, space="PSUM") as mm_ps,
            tc.tile_pool(name="ctx_ps", bufs=1, space="PSUM") as ctx_ps,
            tc.tile_pool(name="sum_ps", bufs=1, space="PSUM") as sum_ps,
        ):
            w_sb = const.tile([P, EC, A], BF)
            wabc_sb = const.tile([P, A], BF)
            pqbc_sb = const.tile([P, BPC * A], BF)
            # PE clock warmup: the HAM gate holds the PE at 1.2GHz until
            # ~3.4us of sustained activity. Burn the initial DMA wait on
            # dummy matmuls so the real stream starts at 2.4GHz. The
            # 4-byte warmout DMA keeps DCE from deleting them. zf's
            # memset is emitted FIRST so nothing delays the warmup.
            # same shape as the real stream (K=128, M=128, N=512):
            # M=1 dummies measurably slowed every later matmul
            zf = const.tile([P, A], BF)
            nc.vector.memset(zf, 0.0)
            # borrows the sum_ps bank (unused until the first
            # finalize ~45us in) so the mm_ps rotation stays intact
            warm_ps = sum_ps.tile([P, A], F32, name="sum")
            for _ in range(N_WARMUP):
                nc.tensor.matmul(warm_ps, zf[:, 0:P], zf, start=True, stop=True)
            warm_sb = const.tile([1, 1], F32)
            nc.vector.tensor_copy(warm_sb, warm_ps[0:1, 0:1])
            nc.gpsimd.dma_start(warmout.ap(), warm_sb)

            # projq/w_att broadcasts via PE ones-matmuls in the warmup
            # dead zone: the tiny rows (5KB) load into partition 0 of a
            # zeroed tile; ones128.T @ row_tile column-sums them onto all
            # 128 partitions. Removes 0.64MB of stride-0 HBM re-reads
            # from the critical DMA ramp entirely.
            row_sb = const.tile([P, (BPC + 1) * A], BF)
            nc.vector.memset(row_sb, 0.0)
            ones128 = const.tile([P, P], BF)
            nc.vector.memset(ones128, 1.0)
            nc.scalar.dma_start(row_sb[0:1, 0 : BPC * A], pq.ap())
            nc.scalar.dma_start(row_sb[0:1, BPC * A :], wa.ap())
            for k in range(BPC + 1):
                bc = mm_ps.tile([P, A], F32, name="mm")
                nc.tensor.matmul(
                    bc,
                    ones128,
                    row_sb[:, k * A : (k + 1) * A],
                    start=True,
                    stop=True,
                )
                dstv = pqbc_sb[:, k * A : (k + 1) * A] if k < BPC else wabc_sb
                nc.vector.tensor_copy(dstv, bc)

            # M=1 stationaries (col_grp=q0) measurably perturb the PE
            # stream, so the ctx/sum matmuls use full-width stationaries
            # padded with zero columns: col 0 carries the real weights,
            # the output's rows 1-127 accumulate zeros. Memsets go on
            # GpSimd: off the warmup's DVE path, not needed until ~30us.
            exppad = const.tile([P, ST, P], BF)
            nc.gpsimd.memset(exppad, 0.0)
            onespad = const.tile([P, P], BF)
            nc.gpsimd.memset(onespad, 0.0)
            nc.gpsimd.memset(onespad[:, 0:1], 1.0)

            # DMA staggering: concurrent DMAs share the SDMA engines
            # round-robin and all complete together at the BW limit, so
            # loads are chained in GROUPS — each group depends on the
            # previous group's last member; members within a group run
            # concurrently. Group size tracks the PE's consumption rate.
            _gate = [None]

            def dma_in_group(engine, dst, src):
                d = engine.dma_start(dst, src)
                if _gate[0] is not None:
                    tile.add_dep_helper(
                        d.ins, _gate[0].ins, reason="stagger input loads"
                    )
                return d

            def next_group(last):
                _gate[0] = last

            state = {}  # batch -> dict(pbs, expbf, cp)

            def emit_ctx(b, st):
                st_ = state[b]
                nc.tensor.matmul(
                    st_["cp"],
                    exppad[:, st, :],
                    st_["pbs"][st],
                    start=(st == 0),
                    stop=(st == ST - 1),
                )

            def finalize_pre(b):
                # denominator chain: emitted BEFORE the last ctx run so
                # the DVE reduce/reciprocal overlap the PE's ctx matmuls
                st_ = state[b]
                sp = sum_ps.tile([P, ST], F32, name="sum")
                nc.tensor.matmul(sp, onespad, st_["expbf"], start=True, stop=True)
                tot = small.tile([1, 1], F32, name="tot")
                nc.vector.tensor_reduce(
                    tot, sp[0:1, :], axis=mybir.AxisListType.X, op=mybir.AluOpType.add
                )
                rcp = small.tile([1, 1], F32, name="rcp")
                nc.vector.reciprocal(rcp, tot)
                st_["rcp"] = rcp

            def finalize(b):
                st_ = state[b]
                rcp = st_["rcp"]
                orow = small.tile([1, A], F32, name="orow")
                # orow = (ctx * 1/total) - bf16(projq), one fused DVE op
                nc.vector.scalar_tensor_tensor(
                    out=orow,
                    in0=st_["cp"][0:1, :],
                    scalar=rcp,
                    in1=pqbc_sb[0:1, b * A : (b + 1) * A],
                    op0=mybir.AluOpType.mult,
                    op1=mybir.AluOpType.subtract,
                )
                nc.sync.dma_start(out.ap()[b : b + 1, :], orow)
                del state[b]

            for b in range(BPC):
                xt_all = xtp.tile([P, NQ, EC, QW], BF, name="xt")
                if b == 0:
                    # G0: {W, x0} -> {x1} -> {x2,x3} -> {x4,x5} -> {x6,x7}
                    nc.scalar.dma_start(w_sb, wh.ap())
                    g = dma_in_group(nc.sync, xt_all[:, 0], xh.ap()[0, :, 0])
                    next_group(g)
                    g = dma_in_group(nc.sync, xt_all[:, 1], xh.ap()[0, :, 1])
                    next_group(g)
                    for q in range(2, NQ, 2):
                        dma_in_group(nc.sync, xt_all[:, q], xh.ap()[0, :, q])
                        g = dma_in_group(nc.sync, xt_all[:, q + 1], xh.ap()[0, :, q + 1])
                        next_group(g)
                else:
                    # halves, each its own chain link: keeps later batches
                    # from stealing packets while batch-0 slices stream in
                    h = NQ // 2
                    g = dma_in_group(nc.scalar, xt_all[:, :h], xh.ap()[b, :, :h])
                    next_group(g)
                    g = dma_in_group(nc.scalar, xt_all[:, h:], xh.ap()[b, :, h:])
                    next_group(g)

                scores = small.tile([P, ST], F32, name="scores")
                expbf = small.tile([P, ST], BF, name="expbf")
                cp = ctx_ps.tile([P, A], F32, name="ctx")
                state[b] = {"pbs": [], "expbf": expbf, "cp": cp}

                for st in range(ST):
                    q, r = st // TPQ, st % TPQ
                    ps = mm_ps.tile([P, A], F32, name="mm")
                    for ec in range(EC):
                        nc.tensor.matmul(
                            ps,
                            xt_all[:, q, ec, r * P : (r + 1) * P],
                            w_sb[:, ec, :],
                            start=(ec == 0),
                            stop=(ec == EC - 1),
                        )
                    # single PSUM reader: drain + bias + bf16 cast in one op
                    pb = pbp.tile([P, A], BF, name="pb")
                    nc.vector.tensor_tensor(
                        out=pb,
                        in0=ps,
                        in1=pqbc_sb[:, b * A : (b + 1) * A],
                        op=mybir.AluOpType.add,
                    )
                    state[b]["pbs"].append(pb)
                    t = tp.tile([P, A], BF, name="t")
                    nc.scalar.activation(t, pb, TANH)
                    # fused (t * w_att) with free-dim accumulation -> scores
                    scr = tp.tile([P, A], BF, name="scr", bufs=2)
                    nc.vector.scalar_tensor_tensor(
                        out=scr,
                        in0=t,
                        scalar=0.0,
                        in1=wabc_sb,
                        op0=mybir.AluOpType.bypass,
                        op1=mybir.AluOpType.mult,
                        accum_out=scores[:, st : st + 1],
                    )
                    nc.scalar.activation(
                        expbf[:, st : st + 1], scores[:, st : st + 1], EXP
                    )
                    nc.vector.tensor_copy(
                        exppad[:, st, 0:1], expbf[:, st : st + 1]
                    )
                    # trailing ctx runs: sts {0-3} after st6, {4-7} after
                    # st10, {8-11} after st14 (grouped: each main<->ctx
                    # switch costs ~190ns on the PE)
                    if st >= 6 and st % CTX_RUN == 2:
                        base = (st - 6) // CTX_RUN * CTX_RUN
                        for k in range(CTX_RUN):
                            emit_ctx(b, base + k)

                    # previous batch's last ctx run + finalize
                    if (b - 1) in state and st == 1:
                        finalize_pre(b - 1)
                        for k in range(ST - CTX_RUN, ST):
                            emit_ctx(b - 1, k)
                        finalize(b - 1)

            # last batch's tail
            finalize_pre(BPC - 1)
            for k in range(ST - CTX_RUN, ST):
                emit_ctx(BPC - 1, k)
            finalize(BPC - 1)

    nc.compile()
    return nc


def make_in_maps(inputs, query, W_in, W_q, w_att):
    bf = ml_dtypes.bfloat16
    x_bf = np.asarray(inputs).astype(bf)  # [B, S, E]
    w_in_bf = np.asarray(W_in).astype(bf)  # [E, A]
    # wh[p, ec, a] = W_in[ec*128+p, a]
    wh_np = np.ascontiguousarray(w_in_bf.reshape(EC, P, A).transpose(1, 0, 2))
    projq = np.asarray(query, dtype=np.float32) @ np.asarray(W_q, dtype=np.float32)
    pq_bf = projq.astype(bf)  # [B, A]
    wa_np = np.ascontiguousarray(np.asarray(w_att).astype(bf).reshape(1, A))

    in_maps = []
    for c in range(NCORES):
        sl = slice(c * BPC, (c + 1) * BPC)
        # xh[b, p, q, ec, qw] = x[b, q*QW+qw, ec*128+p]
        xh_np = np.ascontiguousarray(
            x_bf[sl].reshape(BPC, NQ, QW, EC, P).transpose(0, 4, 1, 3, 2)
        )
        in_maps.append(
            {
                "xh": xh_np,
                "wh": wh_np,
                "pq": np.ascontiguousarray(pq_bf[sl].reshape(1, BPC * A)),
                "wa": wa_np,
            }
        )
    return in_maps


_nc = None


def kernel(inputs, query, W_in, W_q, w_att):
    global _nc
    if _nc is None:
        _nc = build()

    in_maps = make_in_maps(inputs, query, W_in, W_q, w_att)
    res = bass_utils.run_bass_kernel_spmd(_nc, in_maps, core_ids=list(range(NCORES)))
    return np.concatenate([r["out"] for r in res.results], axis=0)


if __name__ == "__main__":
    rng = np.random.default_rng(0)
    ins = {
        "inputs": rng.standard_normal((B, S, E), dtype=np.float32),
        "query": rng.standard_normal((B, Q), dtype=np.float32),
        "W_in": (rng.standard_normal((E, A), dtype=np.float32) / np.sqrt(E)).astype(
            np.float32
        ),
        "W_q": (rng.standard_normal((Q, A), dtype=np.float32) / np.sqrt(Q)).astype(
            np.float32
        ),
        "w_att": (rng.standard_normal((A,), dtype=np.float32) / np.sqrt(A)).astype(
            np.float32
        ),
    }
    got = kernel(**ins)
    print("out shape", got.shape, got.dtype)


# revision 66
# speedup vs baseline: 1.2021x; 1.2021x over previous
"""Bahdanau (additive) attention TRN2 Bass kernel — v5, S-on-partitions.

reference:
    proj_in = einsum("bse,ea->bsa", inputs, W_in)      # [B,S,A]
    proj_q  = (query @ W_q)[:, None, :]                # [B,1,A]
    scores  = einsum("bsa,a->bs", tanh(proj_in+proj_q), w_att)
    weights = softmax(scores, axis=1)
    context = einsum("bs,bsa->ba", weights, proj_in)   # [B,A]

B,S,E,Q,A = 32,2048,1024,1024,512.

Sharding: data-parallel over batch. 8 cores x 4 batches; weights
replicated. proj_q is precomputed on the host (0.05% of FLOPs) and
shipped pre-broadcast across partitions, as is w_att. X and W_in are
shipped in device-native tile layouts so every DMA is 128 descriptors
of multi-KB contiguous runs (HWDGE dispatch ~5ns/descriptor, and the
"small descriptors are slow" HBM penalty never hits).

Device algorithm (per batch; proj tiles come out [s_tile=128, a=512]
with the SEQUENCE dim on partitions):
  - per s_tile (16): 8 e-chunk matmuls accumulate psum[s,a]; one DVE
    tensor_tensor drains psum, fusing the +proj_q bias (partition-
    broadcast tile) and the bf16 cast -> pb.
  - ACT tanh(pb) -> t; one DVE scalar_tensor_tensor computes
    (t * w_att_bcast) with accum_out = scores[:,st] (fused
    multiply+free-dim-reduce: softmax scores land ON partitions).
  - ACT exp -> expbf[:,st] bf16 (no max subtraction: |scores|<=~3).
    No cross-partition broadcast anywhere.
  - context: PE matmuls ctx[1,a] += expbf[:,st].T @ pb(st), emitted in
    RUNS OF EIGHT trailing the main stream (each main<->ctx stationary
    switch costs ~190ns on the PE, so grouping beats interleaving
    singly; the lag keeps the PE from head-blocking on the DVE/ACT
    chain). Denominator = ones.T @ expbf + reduce + reciprocal,
    emitted before the ctx run so the DVE chain hides under it.
  - 16 dummy matmuls (realistic 128x128x512 shape; M=1 dummies
    measurably slow all later matmuls) warm the PE HAM clock gate
    1.2->2.4GHz during the initial DMA wait.
  - out_row = (ctx * 1/total) - bf16(projq) in one fused DVE op; since
    pb = proj_in + bf16(projq), subtracting the same bf16 projq
    cancels the bias exactly. Contiguous 2KB output DMA per batch.
  - batch b's last ctx run + finalize interleave into batch b+1's
    stream; only batch 3's ~3us chain is a tail.
  - Input DMAs are explicitly CHAINED (add_dep_helper): concurrent
    DMAs round-robin per-packet across the 16 SDMA engines and all
    finish together at the BW limit, so an unordered head load gates
    the first matmul on the LAST byte of 5MB. The chain staggers
    arrivals: W, then batch-0 X in eighths, then batches 1-3 whole.
"""

import sys

sys.path.insert(0, "/opt/trn_rl_repo")

import ml_dtypes
import numpy as np

import concourse.bass as bass
import concourse.tile as tile
from concourse import bacc, bass_utils, mybir

B, S, E, Q, A = 32, 2048, 1024, 1024, 512
NCORES = 8
BPC = B // NCORES  # batches per core
P = 128
EC = E // P  # 8 e-chunks (contraction)
ST = S // P  # 16 s-tiles per batch (output partition tiles)
NQ = 8  # batch-0 load slices (s-eighths)
QW = S // NQ  # 256
TPQ = ST // NQ  # s_tiles per load slice = 2
CTX_RUN = 8  # ctx matmuls emitted in runs of this many
N_WARMUP = 14  # dummy PE-warmup matmuls before the real stream

BF = mybir.dt.bfloat16
F32 = mybir.dt.float32
TANH = mybir.ActivationFunctionType.Tanh
EXP = mybir.ActivationFunctionType.Exp


def build():
    nc = bacc.Bacc("TRN2", target_bir_lowering=False, debug=False)

    # host-tiled layouts: xh[b, p, q, ec, qw], wh[p, ec, a]
    xh = nc.dram_tensor("xh", [BPC, P, NQ, EC, QW], BF, kind="ExternalInput")
    wh = nc.dram_tensor("wh", [P, EC, A], BF, kind="ExternalInput")
    pq = nc.dram_tensor("pq", [1, BPC * A], BF, kind="ExternalInput")
    wa = nc.dram_tensor("wa", [1, A], BF, kind="ExternalInput")
    out = nc.dram_tensor("out", [BPC, A], F32, kind="ExternalOutput")
    warmout = nc.dram_tensor("warmout", [1, 1], F32, kind="ExternalOutput")

    with tile.TileContext(nc) as tc:
        with (
            tc.tile_pool(name="const", bufs=1) as const,
            tc.tile_pool(name="xtp", bufs=2) as xtp,
            tc.tile_pool(name="pbp", bufs=13) as pbp,
            tc.tile_pool(name="tp", bufs=3) as tp,
            tc.tile_pool(name="small", bufs=2) as small,
            tc.tile_pool(name="mm_ps", bufs=6, space="PSUM") as mm_ps,
            tc.tile_pool(name="ctx_ps", bufs=1, space="PSUM") as ctx_ps,
            tc.tile_pool(name="sum_ps", bufs=1, space="PSUM") as sum_ps,
        ):
            w_sb = const.tile([P, EC, A], BF)
            wabc_sb = const.tile([P, A], BF)
            pqbc_sb = const.tile([P, BPC * A], BF)
            # PE clock warmup: the HAM gate holds the PE at 1.2GHz until
            # ~3.4us of sustained activity. Burn the initial DMA wait on
            # dummy matmuls so the real stream starts at 2.4GHz. The
            # 4-byte warmout DMA keeps DCE from deleting them. zf's
            # memset is emitted FIRST so nothing delays the warmup.
            # same shape as the real stream (K=128, M=128, N=512):
            # M=1 dummies measurably slowed every later matmul
            zf = const.tile([P, A], BF)
            nc.vector.memset(zf, 0.0)
            # borrows the sum_ps bank (unused until the first
            # finalize ~45us in) so the mm_ps rotation stays intact
            warm_ps = sum_ps.tile([P, A], F32, name="sum")
            for _ in range(N_WARMUP):
                nc.tensor.matmul(warm_ps, zf[:, 0:P], zf, start=True, stop=True)
            warm_sb = const.tile([1, 1], F32)
            nc.vector.tensor_copy(warm_sb, warm_ps[0:1, 0:1])
            nc.gpsimd.dma_start(warmout.ap(), warm_sb)



            # M=1 stationaries (col_grp=q0) measurably perturb the PE
            # stream, so the ctx/sum matmuls use full-width stationaries
            # padded with zero columns: col 0 carries the real weights,
            # the output's rows 1-127 accumulate zeros. Memsets go on
            # GpSimd: off the warmup's DVE path, not needed until ~30us.
            exppad = const.tile([P, ST, P], BF)
            nc.gpsimd.memset(exppad, 0.0)
            onespad = const.tile([P, P], BF)
            nc.gpsimd.memset(onespad, 0.0)
            nc.gpsimd.memset(onespad[:, 0:1], 1.0)

            # DMA staggering: concurrent DMAs share the SDMA engines
            # round-robin and all complete together at the BW limit, so
            # loads are chained in GROUPS — each group depends on the
            # previous group's last member; members within a group run
            # concurrently. Group size tracks the PE's consumption rate.
            _gate = [None]

            def dma_in_group(engine, dst, src):
                d = engine.dma_start(dst, src)
                if _gate[0] is not None:
                    tile.add_dep_helper(
                        d.ins, _gate[0].ins, reason="stagger input loads"
                    )
                return d

            def next_group(last):
                _gate[0] = last

            state = {}  # batch -> dict(pbs, expbf, cp)

            def emit_ctx(b, st):
                st_ = state[b]
                nc.tensor.matmul(
                    st_["cp"],
                    exppad[:, st, :],
                    st_["pbs"][st],
                    start=(st == 0),
                    stop=(st == ST - 1),
                )

            def finalize_pre(b):
                # denominator chain: emitted BEFORE the last ctx run so
                # the DVE reduce/reciprocal overlap the PE's ctx matmuls
                st_ = state[b]
                sp = sum_ps.tile([P, ST], F32, name="sum")
                nc.tensor.matmul(sp, onespad, st_["expbf"], start=True, stop=True)
                tot = small.tile([1, 1], F32, name="tot")
                nc.vector.tensor_reduce(
                    tot, sp[0:1, :], axis=mybir.AxisListType.X, op=mybir.AluOpType.add
                )
                rcp = small.tile([1, 1], F32, name="rcp")
                nc.vector.reciprocal(rcp, tot)
                st_["rcp"] = rcp

            def finalize(b):
                st_ = state[b]
                rcp = st_["rcp"]
                orow = small.tile([1, A], F32, name="orow")
                # orow = (ctx * 1/total) - bf16(projq), one fused DVE op
                nc.vector.scalar_tensor_tensor(
                    out=orow,
                    in0=st_["cp"][0:1, :],
                    scalar=rcp,
                    in1=pqbc_sb[0:1, b * A : (b + 1) * A],
                    op0=mybir.AluOpType.mult,
                    op1=mybir.AluOpType.subtract,
                )
                nc.sync.dma_start(out.ap()[b : b + 1, :], orow)
                del state[b]

            for b in range(BPC):
                xt_all = xtp.tile([P, NQ, EC, QW], BF, name="xt")
                if b == 0:
                    # G0: {W, x0} -> {x1, bcasts} -> {x2,x3} -> {x4,x5}
                    # -> {x6,x7}. pqbc must land by ~16us or the DVE bias
                    # chain starves and the first ctx run stalls the PE.
                    nc.scalar.dma_start(w_sb, wh.ap())
                    g = dma_in_group(nc.sync, xt_all[:, 0], xh.ap()[0, :, 0])
                    next_group(g)
                    g = dma_in_group(nc.sync, xt_all[:, 1], xh.ap()[0, :, 1])
                    dma_in_group(
                        nc.sync,
                        pqbc_sb,
                        bass.AP(tensor=pq, offset=0, ap=[[0, P], [1, BPC * A]]),
                    )
                    dma_in_group(
                        nc.sync,
                        wabc_sb,
                        bass.AP(tensor=wa, offset=0, ap=[[0, P], [1, A]]),
                    )
                    next_group(g)
                    for q in range(2, NQ, 2):
                        dma_in_group(nc.sync, xt_all[:, q], xh.ap()[0, :, q])
                        g = dma_in_group(nc.sync, xt_all[:, q + 1], xh.ap()[0, :, q + 1])
                        next_group(g)
                else:
                    # halves, each its own chain link: keeps later batches
                    # from stealing packets while batch-0 slices stream in
                    h = NQ // 2
                    g = dma_in_group(nc.scalar, xt_all[:, :h], xh.ap()[b, :, :h])
                    next_group(g)
                    g = dma_in_group(nc.scalar, xt_all[:, h:], xh.ap()[b, :, h:])
                    next_group(g)

                scores = small.tile([P, ST], F32, name="scores")
                expbf = small.tile([P, ST], BF, name="expbf")
                cp = ctx_ps.tile([P, A], F32, name="ctx")
                state[b] = {"pbs": [], "expbf": expbf, "cp": cp}

                for st in range(ST):
                    q, r = st // TPQ, st % TPQ
                    ps = mm_ps.tile([P, A], F32, name="mm")
                    for ec in range(EC):
                        nc.tensor.matmul(
                            ps,
                            xt_all[:, q, ec, r * P : (r + 1) * P],
                            w_sb[:, ec, :],
                            start=(ec == 0),
                            stop=(ec == EC - 1),
                        )
                    # single PSUM reader: drain + bias + bf16 cast in one op
                    pb = pbp.tile([P, A], BF, name="pb")
                    nc.vector.tensor_tensor(
                        out=pb,
                        in0=ps,
                        in1=pqbc_sb[:, b * A : (b + 1) * A],
                        op=mybir.AluOpType.add,
                    )
                    state[b]["pbs"].append(pb)
                    t = tp.tile([P, A], BF, name="t")
                    nc.scalar.activation(t, pb, TANH)
                    # fused (t * w_att) with free-dim accumulation -> scores
                    scr = tp.tile([P, A], BF, name="scr", bufs=2)
                    nc.vector.scalar_tensor_tensor(
                        out=scr,
                        in0=t,
                        scalar=0.0,
                        in1=wabc_sb,
                        op0=mybir.AluOpType.bypass,
                        op1=mybir.AluOpType.mult,
                        accum_out=scores[:, st : st + 1],
                    )
                    nc.scalar.activation(
                        expbf[:, st : st + 1], scores[:, st : st + 1], EXP
                    )
                    nc.vector.tensor_copy(
                        exppad[:, st, 0:1], expbf[:, st : st + 1]
                    )
                    # trailing ctx runs: sts {0-3} after st6, {4-7} after
                    # st10, {8-11} after st14 (grouped: each main<->ctx
                    # switch costs ~190ns on the PE)
                    if st >= 6 and st % CTX_RUN == 2:
                        base = (st - 6) // CTX_RUN * CTX_RUN
                        for k in range(CTX_RUN):
                            emit_ctx(b, base + k)

                    # previous batch's last ctx run + finalize
                    if (b - 1) in state and st == 1:
                        finalize_pre(b - 1)
                        for k in range(ST - CTX_RUN, ST):
                            emit_ctx(b - 1, k)
                        finalize(b - 1)

            # last batch's tail
            finalize_pre(BPC - 1)
            for k in range(ST - CTX_RUN, ST):
                emit_ctx(BPC - 1, k)
            finalize(BPC - 1)

    nc.compile()
    return nc


def make_in_maps(inputs, query, W_in, W_q, w_att):
    bf = ml_dtypes.bfloat16
    x_bf = np.asarray(inputs).astype(bf)  # [B, S, E]
    w_in_bf = np.asarray(W_in).astype(bf)  # [E, A]
    # wh[p, ec, a] = W_in[ec*128+p, a]
    wh_np = np.ascontiguousarray(w_in_bf.reshape(EC, P, A).transpose(1, 0, 2))
    projq = np.asarray(query, dtype=np.float32) @ np.asarray(W_q, dtype=np.float32)
    pq_bf = projq.astype(bf)  # [B, A]
    wa_np = np.ascontiguousarray(np.asarray(w_att).astype(bf).reshape(1, A))

    in_maps = []
    for c in range(NCORES):
        sl = slice(c * BPC, (c + 1) * BPC)
        # xh[b, p, q, ec, qw] = x[b, q*QW+qw, ec*128+p]
        xh_np = np.ascontiguousarray(
            x_bf[sl].reshape(BPC, NQ, QW, EC, P).transpose(0, 4, 1, 3, 2)
        )
        in_maps.append(
            {
                "xh": xh_np,
                "wh": wh_np,
                "pq": np.ascontiguousarray(pq_bf[sl].reshape(1, BPC * A)),
                "wa": wa_np,
            }
        )
    return in_maps


_nc = None


def kernel(inputs, query, W_in, W_q, w_att):
    global _nc
    if _nc is None:
        _nc = build()

    in_maps = make_in_maps(inputs, query, W_in, W_q, w_att)
    res = bass_utils.run_bass_kernel_spmd(_nc, in_maps, core_ids=list(range(NCORES)))
    return np.concatenate([r["out"] for r in res.results], axis=0)


if __name__ == "__main__":
    rng = np.random.default_rng(0)
    ins = {
        "inputs": rng.standard_normal((B, S, E), dtype=np.float32),
        "query": rng.standard_normal((B, Q), dtype=np.float32),
        "W_in": (rng.standard_normal((E, A), dtype=np.float32) / np.sqrt(E)).astype(
            np.float32
        ),
        "W_q": (rng.standard_normal((Q, A), dtype=np.float32) / np.sqrt(Q)).astype(
            np.float32
        ),
        "w_att": (rng.standard_normal((A,), dtype=np.float32) / np.sqrt(A)).astype(
            np.float32
        ),
    }
    got = kernel(**ins)
    print("out shape", got.shape, got.dtype)


# revision 68
# speedup vs baseline: 1.2067x; 1.0038x over previous
"""Bahdanau (additive) attention TRN2 Bass kernel — v5, S-on-partitions.

reference:
    proj_in = einsum("bse,ea->bsa", inputs, W_in)      # [B,S,A]
    proj_q  = (query @ W_q)[:, None, :]                # [B,1,A]
    scores  = einsum("bsa,a->bs", tanh(proj_in+proj_q), w_att)
    weights = softmax(scores, axis=1)
    context = einsum("bs,bsa->ba", weights, proj_in)   # [B,A]

B,S,E,Q,A = 32,2048,1024,1024,512.

Sharding: data-parallel over batch. 8 cores x 4 batches; weights
replicated. proj_q is precomputed on the host (0.05% of FLOPs) and
shipped pre-broadcast across partitions, as is w_att. X and W_in are
shipped in device-native tile layouts so every DMA is 128 descriptors
of multi-KB contiguous runs (HWDGE dispatch ~5ns/descriptor, and the
"small descriptors are slow" HBM penalty never hits).

Device algorithm (per batch; proj tiles come out [s_tile=128, a=512]
with the SEQUENCE dim on partitions):
  - per s_tile (16): 8 e-chunk matmuls accumulate psum[s,a]; one DVE
    tensor_tensor drains psum, fusing the +proj_q bias (partition-
    broadcast tile) and the bf16 cast -> pb.
  - ACT tanh(pb) -> t; one DVE scalar_tensor_tensor computes
    (t * w_att_bcast) with accum_out = scores[:,st] (fused
    multiply+free-dim-reduce: softmax scores land ON partitions).
  - ACT exp -> expbf[:,st] bf16 (no max subtraction: |scores|<=~3).
    No cross-partition broadcast anywhere.
  - context: PE matmuls ctx[1,a] += expbf[:,st].T @ pb(st), emitted in
    RUNS OF EIGHT trailing the main stream (each main<->ctx stationary
    switch costs ~190ns on the PE, so grouping beats interleaving
    singly; the lag keeps the PE from head-blocking on the DVE/ACT
    chain). Denominator = ones.T @ expbf + reduce + reciprocal,
    emitted before the ctx run so the DVE chain hides under it.
  - 16 dummy matmuls (realistic 128x128x512 shape; M=1 dummies
    measurably slow all later matmuls) warm the PE HAM clock gate
    1.2->2.4GHz during the initial DMA wait.
  - out_row = (ctx * 1/total) - bf16(projq) in one fused DVE op; since
    pb = proj_in + bf16(projq), subtracting the same bf16 projq
    cancels the bias exactly. Contiguous 2KB output DMA per batch.
  - batch b's last ctx run + finalize interleave into batch b+1's
    stream; only batch 3's ~3us chain is a tail.
  - Input DMAs are explicitly CHAINED (add_dep_helper): concurrent
    DMAs round-robin per-packet across the 16 SDMA engines and all
    finish together at the BW limit, so an unordered head load gates
    the first matmul on the LAST byte of 5MB. The chain staggers
    arrivals: W, then batch-0 X in eighths, then batches 1-3 whole.
"""

import sys

sys.path.insert(0, "/opt/trn_rl_repo")

import ml_dtypes
import numpy as np

import concourse.bass as bass
import concourse.tile as tile
from concourse import bacc, bass_utils, mybir

B, S, E, Q, A = 32, 2048, 1024, 1024, 512
NCORES = 8
BPC = B // NCORES  # batches per core
P = 128
EC = E // P  # 8 e-chunks (contraction)
ST = S // P  # 16 s-tiles per batch (output partition tiles)
NQ = 8  # batch-0 load slices (s-eighths)
QW = S // NQ  # 256
TPQ = ST // NQ  # s_tiles per load slice = 2
CTX_RUN = 8  # ctx matmuls emitted in runs of this many
N_WARMUP = 14  # dummy PE-warmup matmuls before the real stream

BF = mybir.dt.bfloat16
F32 = mybir.dt.float32
TANH = mybir.ActivationFunctionType.Tanh
EXP = mybir.ActivationFunctionType.Exp


def build():
    nc = bacc.Bacc("TRN2", target_bir_lowering=False, debug=False)

    # host-tiled layouts: xh[b, p, q, ec, qw], wh[p, ec, a]
    xh = nc.dram_tensor("xh", [BPC, P, NQ, EC, QW], BF, kind="ExternalInput")
    wh = nc.dram_tensor("wh", [P, EC, A], BF, kind="ExternalInput")
    pq = nc.dram_tensor("pq", [1, BPC * A], BF, kind="ExternalInput")
    wa = nc.dram_tensor("wa", [1, A], BF, kind="ExternalInput")
    out = nc.dram_tensor("out", [BPC, A], F32, kind="ExternalOutput")
    warmout = nc.dram_tensor("warmout", [1, 1], F32, kind="ExternalOutput")

    with tile.TileContext(nc) as tc:
        with (
            tc.tile_pool(name="const", bufs=1) as const,
            tc.tile_pool(name="xtp", bufs=2) as xtp,
            tc.tile_pool(name="pbp", bufs=13) as pbp,
            tc.tile_pool(name="tp", bufs=3) as tp,
            tc.tile_pool(name="small", bufs=2) as small,
            tc.tile_pool(name="mm_ps", bufs=6, space="PSUM") as mm_ps,
            tc.tile_pool(name="ctx_ps", bufs=1, space="PSUM") as ctx_ps,
            tc.tile_pool(name="sum_ps", bufs=1, space="PSUM") as sum_ps,
        ):
            w_sb = const.tile([P, EC, A], BF)
            wabc_sb = const.tile([P, A], BF)
            pqbc_sb = const.tile([P, BPC * A], BF)
            # PE clock warmup: the HAM gate holds the PE at 1.2GHz until
            # ~3.4us of sustained activity. Burn the initial DMA wait on
            # dummy matmuls so the real stream starts at 2.4GHz. The
            # 4-byte warmout DMA keeps DCE from deleting them. zf's
            # memset is emitted FIRST so nothing delays the warmup.
            # same shape as the real stream (K=128, M=128, N=512):
            # M=1 dummies measurably slowed every later matmul
            zf = const.tile([P, A], BF)
            nc.vector.memset(zf, 0.0)
            # borrows the sum_ps bank (unused until the first
            # finalize ~45us in) so the mm_ps rotation stays intact
            warm_ps = sum_ps.tile([P, A], F32, name="sum")
            for _ in range(N_WARMUP):
                nc.tensor.matmul(warm_ps, zf[:, 0:P], zf, start=True, stop=True)
            warm_sb = const.tile([1, 1], F32)
            nc.vector.tensor_copy(warm_sb, warm_ps[0:1, 0:1])
            nc.gpsimd.dma_start(warmout.ap(), warm_sb)



            # M=1 stationaries (col_grp=q0) measurably perturb the PE
            # stream, so the ctx/sum matmuls use full-width stationaries
            # padded with zero columns: col 0 carries the real weights,
            # the output's rows 1-127 accumulate zeros. Memsets go on
            # GpSimd: off the warmup's DVE path, not needed until ~30us.
            exppad = const.tile([P, ST, P], BF)
            nc.gpsimd.memset(exppad, 0.0)
            onespad = const.tile([P, P], BF)
            nc.gpsimd.memset(onespad, 0.0)
            nc.gpsimd.memset(onespad[:, 0:1], 1.0)

            # DMA staggering: concurrent DMAs share the SDMA engines
            # round-robin and all complete together at the BW limit, so
            # loads are chained in GROUPS — each group depends on the
            # previous group's last member; members within a group run
            # concurrently. Group size tracks the PE's consumption rate.
            _gate = [None]

            def dma_in_group(engine, dst, src):
                d = engine.dma_start(dst, src)
                if _gate[0] is not None:
                    tile.add_dep_helper(
                        d.ins, _gate[0].ins, reason="stagger input loads"
                    )
                return d

            def next_group(last):
                _gate[0] = last

            state = {}  # batch -> dict(pbs, expbf, cp)

            def emit_ctx(b, st):
                st_ = state[b]
                nc.tensor.matmul(
                    st_["cp"],
                    exppad[:, st, :],
                    st_["pbs"][st],
                    start=(st == 0),
                    stop=(st == ST - 1),
                )

            def finalize_pre(b):
                # denominator chain: emitted BEFORE the last ctx run so
                # the DVE reduce/reciprocal overlap the PE's ctx matmuls
                st_ = state[b]
                sp = sum_ps.tile([P, ST], F32, name="sum")
                rhs = exppad[:, :, 0:1] if b == BPC - 1 else st_["expbf"]
                nc.tensor.matmul(sp, onespad, rhs, start=True, stop=True)
                tot = small.tile([1, 1], F32, name="tot")
                nc.vector.tensor_reduce(
                    tot, sp[0:1, :], axis=mybir.AxisListType.X, op=mybir.AluOpType.add
                )
                rcp = small.tile([1, 1], F32, name="rcp")
                nc.vector.reciprocal(rcp, tot)
                st_["rcp"] = rcp

            def finalize(b):
                st_ = state[b]
                rcp = st_["rcp"]
                orow = small.tile([1, A], F32, name="orow")
                # orow = (ctx * 1/total) - bf16(projq), one fused DVE op
                nc.vector.scalar_tensor_tensor(
                    out=orow,
                    in0=st_["cp"][0:1, :],
                    scalar=rcp,
                    in1=pqbc_sb[0:1, b * A : (b + 1) * A],
                    op0=mybir.AluOpType.mult,
                    op1=mybir.AluOpType.subtract,
                )
                nc.sync.dma_start(out.ap()[b : b + 1, :], orow)
                del state[b]

            for b in range(BPC):
                xt_all = xtp.tile([P, NQ, EC, QW], BF, name="xt")
                if b == 0:
                    # G0: {W, x0} -> {x1, bcasts} -> {x2,x3} -> {x4,x5}
                    # -> {x6,x7}. pqbc must land by ~16us or the DVE bias
                    # chain starves and the first ctx run stalls the PE.
                    nc.scalar.dma_start(w_sb, wh.ap())
                    g = dma_in_group(nc.sync, xt_all[:, 0], xh.ap()[0, :, 0])
                    next_group(g)
                    g = dma_in_group(nc.sync, xt_all[:, 1], xh.ap()[0, :, 1])
                    dma_in_group(
                        nc.sync,
                        pqbc_sb,
                        bass.AP(tensor=pq, offset=0, ap=[[0, P], [1, BPC * A]]),
                    )
                    dma_in_group(
                        nc.sync,
                        wabc_sb,
                        bass.AP(tensor=wa, offset=0, ap=[[0, P], [1, A]]),
                    )
                    next_group(g)
                    for q in range(2, NQ, 2):
                        dma_in_group(nc.sync, xt_all[:, q], xh.ap()[0, :, q])
                        g = dma_in_group(nc.sync, xt_all[:, q + 1], xh.ap()[0, :, q + 1])
                        next_group(g)
                else:
                    # halves, each its own chain link: keeps later batches
                    # from stealing packets while batch-0 slices stream in
                    h = NQ // 2
                    g = dma_in_group(nc.scalar, xt_all[:, :h], xh.ap()[b, :, :h])
                    next_group(g)
                    g = dma_in_group(nc.scalar, xt_all[:, h:], xh.ap()[b, :, h:])
                    next_group(g)

                scores = small.tile([P, ST], F32, name="scores")
                expbf = small.tile([P, ST], BF, name="expbf")
                cp = ctx_ps.tile([P, A], F32, name="ctx")
                state[b] = {"pbs": [], "expbf": expbf, "cp": cp}

                for st in range(ST):
                    q, r = st // TPQ, st % TPQ
                    ps = mm_ps.tile([P, A], F32, name="mm")
                    for ec in range(EC):
                        nc.tensor.matmul(
                            ps,
                            xt_all[:, q, ec, r * P : (r + 1) * P],
                            w_sb[:, ec, :],
                            start=(ec == 0),
                            stop=(ec == EC - 1),
                        )
                    # single PSUM reader: drain + bias + bf16 cast in one op
                    pb = pbp.tile([P, A], BF, name="pb")
                    nc.vector.tensor_tensor(
                        out=pb,
                        in0=ps,
                        in1=pqbc_sb[:, b * A : (b + 1) * A],
                        op=mybir.AluOpType.add,
                    )
                    state[b]["pbs"].append(pb)
                    t = tp.tile([P, A], BF, name="t")
                    nc.scalar.activation(t, pb, TANH)
                    # fused (t * w_att) with free-dim accumulation -> scores
                    scr = tp.tile([P, A], BF, name="scr", bufs=2)
                    nc.vector.scalar_tensor_tensor(
                        out=scr,
                        in0=t,
                        scalar=0.0,
                        in1=wabc_sb,
                        op0=mybir.AluOpType.bypass,
                        op1=mybir.AluOpType.mult,
                        accum_out=scores[:, st : st + 1],
                    )
                    if b == BPC - 1:
                        # last batch: exp straight into the padded ctx
                        # stationary (no later batch overwrites it, so
                        # the copy + its sem hop leave the tail chain)
                        nc.scalar.activation(
                            exppad[:, st, 0:1], scores[:, st : st + 1], EXP
                        )
                    else:
                        nc.scalar.activation(
                            expbf[:, st : st + 1], scores[:, st : st + 1], EXP
                        )
                        nc.vector.tensor_copy(
                            exppad[:, st, 0:1], expbf[:, st : st + 1]
                        )
                    # trailing ctx runs: sts {0-3} after st6, {4-7} after
                    # st10, {8-11} after st14 (grouped: each main<->ctx
                    # switch costs ~190ns on the PE)
                    if st >= 6 and st % CTX_RUN == 2:
                        base = (st - 6) // CTX_RUN * CTX_RUN
                        for k in range(CTX_RUN):
                            emit_ctx(b, base + k)

                    # previous batch's last ctx run + finalize
                    if (b - 1) in state and st == 1:
                        finalize_pre(b - 1)
                        for k in range(ST - CTX_RUN, ST):
                            emit_ctx(b - 1, k)
                        finalize(b - 1)

            # last batch's tail
            finalize_pre(BPC - 1)
            for k in range(ST - CTX_RUN, ST):
                emit_ctx(BPC - 1, k)
            finalize(BPC - 1)

    nc.compile()
    return nc


def make_in_maps(inputs, query, W_in, W_q, w_att):
    bf = ml_dtypes.bfloat16
    x_bf = np.asarray(inputs).astype(bf)  # [B, S, E]
    w_in_bf = np.asarray(W_in).astype(bf)  # [E, A]
    # wh[p, ec, a] = W_in[ec*128+p, a]
    wh_np = np.ascontiguousarray(w_in_bf.reshape(EC, P, A).transpose(1, 0, 2))
    projq = np.asarray(query, dtype=np.float32) @ np.asarray(W_q, dtype=np.float32)
    pq_bf = projq.astype(bf)  # [B, A]
    wa_np = np.ascontiguousarray(np.asarray(w_att).astype(bf).reshape(1, A))

    in_maps = []
    for c in range(NCORES):
        sl = slice(c * BPC, (c + 1) * BPC)
        # xh[b, p, q, ec, qw] = x[b, q*QW+qw, ec*128+p]
        xh_np = np.ascontiguousarray(
            x_bf[sl].reshape(BPC, NQ, QW, EC, P).transpose(0, 4, 1, 3, 2)
        )
        in_maps.append(
            {
                "xh": xh_np,
                "wh": wh_np,
                "pq": np.ascontiguousarray(pq_bf[sl].reshape(1, BPC * A)),
                "wa": wa_np,
            }
        )
    return in_maps


_nc = None


def kernel(inputs, query, W_in, W_q, w_att):
    global _nc
    if _nc is None:
        _nc = build()

    in_maps = make_in_maps(inputs, query, W_in, W_q, w_att)
    res = bass_utils.run_bass_kernel_spmd(_nc, in_maps, core_ids=list(range(NCORES)))
    return np.concatenate([r["out"] for r in res.results], axis=0)


if __name__ == "__main__":
    rng = np.random.default_rng(0)
    ins = {
        "inputs": rng.standard_normal((B, S, E), dtype=np.float32),
        "query": rng.standard_normal((B, Q), dtype=np.float32),
        "W_in": (rng.standard_normal((E, A), dtype=np.float32) / np.sqrt(E)).astype(
            np.float32
        ),
        "W_q": (rng.standard_normal((Q, A), dtype=np.float32) / np.sqrt(Q)).astype(
            np.float32
        ),
        "w_att": (rng.standard_normal((A,), dtype=np.float32) / np.sqrt(A)).astype(
            np.float32
        ),
    }
    got = kernel(**ins)
    print("out shape", got.shape, got.dtype)
